# revision 9
# baseline (speedup 1.0000x reference)
"""Trainium2 Bass kernel for nn_DenseExpert (MoE dense-expert gated blend).

Math (full problem, B=8192, E=8, U=512, D=512):
    h[b,e,u] = sum_d x[b,d] * alpha[e,u,d]
    r[b,u]   = sum_e g[b,e] * h[b,e,u] + sum_e g[b,e] * beta[e,u]

Strategy:
  - Data-parallel over batch B across 8 NeuronCores (1024 rows each);
    alpha/beta replicated. No collectives.
  - Matmul operands in bf16 (fp8 DoubleRow faults the exec unit on this
    toolchain; full-fp8 also measured 3.3e-2 rel err, over the 2e-2
    budget). PE streaming 131k cols @2.4GHz = 54.6us is the per-core
    floor.
  - All DMAs use host-prepared layouts contiguous per SBUF partition;
    x and alpha[0] are packed per k-chunk into one tensor (xa0) so the
    serial DMA-issue queue (0.65us per issue) doesn't starve expert 0's
    tail chunks (the issue queue cost expert 0 ~3.4us in the previous
    version).
  - Gated reduction pipelined across the idle elementwise engines:
      ACT: t = h_e * g[:,e]  (per-partition scale, PSUM f32 -> SBUF bf16)
      DVE: acc += t          (all-bf16 tensor_tensor, 2x_1p mode)
    bias = g @ beta precomputed on host, DMA-preloaded into the bf16
    accumulator.
  - Warmup matmuls on a zeroed dummy tile ramp the PE clock out of its
    low p-state while the first input DMAs land.
  - Output written bf16 in [P, M, U] layout (host transposes/upcasts);
    batch tiles run in three phases (m0-3, m4-6, m7) so output DMA
    overlaps compute and PSUM banks cycle 4+4 between adjacent experts.
"""

import os

# Reset cores on first device open: clears lingering degraded DGE/DVFS
# state (observed to cost ~14us/run until reset; recommended by the
# platform docs for post-fault recovery). No-op on a healthy device.
os.environ.setdefault("NEURON_RT_RESET_CORES", "1")

import numpy as np
from contextlib import ExitStack

try:
    import concourse.bass as bass
except ImportError:  # fallback if concourse isn't on the default path
    import sys

    sys.path.insert(0, "/opt/trn_rl_repo")
    import concourse.bass as bass
from concourse import bacc

import concourse.mybir as mybir
import concourse.tile as tile
from concourse.bass_utils import run_bass_kernel_spmd

B, E, U, D = 8192, 8, 512, 512
N_CORES = 8
BC = B // N_CORES  # 1024 batch rows per core
P = 128
M_TILES = BC // P  # 8 batch tiles per core
K_TILES = D // P  # 4 contraction chunks
F32 = mybir.dt.float32
BF16 = mybir.dt.bfloat16

_NC_CACHE = {}
last_results = None  # BassKernelResults of the most recent run (for test harness)

PHASES = [(0, 4), (4, 7), (7, 8)]
N_WARMUP = 14  # dummy matmuls bridge until the first input chunk lands
# (~12.3us): an idle gap before the first real matmul resets the PE
# p-state and costs ~2us of half-clock execution at the head. Emitted
# as ONE accumulation group — separate start/stop groups at this count
# fault the exec unit.


def _build_nc():
    nc = bacc.Bacc("TRN2", target_bir_lowering=False, debug=False)

    # host-prepared layouts, all contiguous per partition line
    xa0 = nc.dram_tensor(
        "xa0", [K_TILES, P, BC + U], BF16, kind="ExternalInput"
    ).ap()
    gP = nc.dram_tensor("gP", [P, M_TILES, E], F32, kind="ExternalInput").ap()
    biasP = nc.dram_tensor("biasP", [P, M_TILES, U], BF16, kind="ExternalInput").ap()
    alphaP = nc.dram_tensor(
        "alphaP", [E, P, K_TILES, U], BF16, kind="ExternalInput"
    ).ap()
    out = nc.dram_tensor("out", [P, M_TILES, U], BF16, kind="ExternalOutput").ap()

    add = mybir.AluOpType.add
    Copy = mybir.ActivationFunctionType.Copy

    with tile.TileContext(nc) as tc, ExitStack() as ctx:
        sml_pool = ctx.enter_context(tc.tile_pool(name="sml", bufs=1))
        at_pool = ctx.enter_context(tc.tile_pool(name="at", bufs=E - 1))
        acc_pool = ctx.enter_context(tc.tile_pool(name="acc", bufs=1))
        t_pool = ctx.enter_context(tc.tile_pool(name="tst", bufs=6))
        ps_pool = ctx.enter_context(tc.tile_pool(name="ps", bufs=8, space="PSUM"))

        # ---- PE warmup: ramp the clock on a zeroed dummy while DMAs land
        dummy = sml_pool.tile([P, U], BF16, tag="dummy", name="dummy")
        nc.gpsimd.memset(dummy[:], 0)
        ps_warm = ps_pool.tile([P, U], F32, tag="ps", name="ps_warm")
        for w in range(N_WARMUP):
            nc.tensor.matmul(
                ps_warm[:],
                dummy[:, :P],
                dummy[:],
                start=(w == 0),
                stop=(w == N_WARMUP - 1),
            )

        # ---- DMA issue order: packed x|alpha0 k-chunks first, then
        # alpha1, g, bias, remaining experts behind.
        xa0s = []
        for k in range(K_TILES):
            t = sml_pool.tile([P, BC + U], BF16, tag=f"xa0{k}", name=f"xa0{k}")
            nc.sync.dma_start(t[:], xa0[k])
            xa0s.append(t)
        ats = [None] + [
            at_pool.tile([P, K_TILES, U], BF16, tag="at", name=f"at{e}")
            for e in range(1, E)
        ]
        nc.sync.dma_start(ats[1][:], alphaP[1])
        g_t = sml_pool.tile([P, M_TILES, E], F32, tag="g", name="gt")
        nc.sync.dma_start(g_t[:], gP[:, :, :])
        acc_t = acc_pool.tile([P, M_TILES, U], BF16, tag="acc", name="acc")
        nc.sync.dma_start(acc_t[:], biasP[:, :, :])  # bias preload
        for e in range(2, E):
            nc.sync.dma_start(ats[e][:], alphaP[e])

        # ---- experts, phased over batch tiles so output writes overlap
        # compute ----
        for a, b in PHASES:
            for e in range(E):
                pes = {}
                for m in range(a, b):
                    pes[m] = ps_pool.tile([P, U], F32, tag="ps", name=f"pe{e}_{m}")
                for k in range(K_TILES):
                    rhs = xa0s[k][:, BC:] if e == 0 else ats[e][:, k, :]
                    for m in range(a, b):
                        nc.tensor.matmul(
                            pes[m][:],
                            xa0s[k][:, bass.ts(m, P)],
                            rhs,
                            start=(k == 0),
                            stop=(k == K_TILES - 1),
                        )
                for m in range(a, b):
                    # ACT: t = h_e * g[:,e]  (PSUM f32 -> SBUF bf16)
                    t_t = t_pool.tile([P, U], BF16, tag="t", name=f"t{e}_{m}")
                    nc.scalar.activation(
                        t_t[:], pes[m][:], Copy, scale=g_t[:, m, e : e + 1]
                    )
                    # DVE: acc += t  (all-bf16 -> 2x mode)
                    nc.vector.tensor_tensor(
                        acc_t[:, m, :], acc_t[:, m, :], t_t[:], op=add
                    )
            if b == M_TILES:
                # tail phase: two half-width transfers overlap, shaving
                # the critical-path output write
                nc.sync.dma_start(out[:, a:b, : U // 2], acc_t[:, a:b, : U // 2])
                nc.sync.dma_start(out[:, a:b, U // 2 :], acc_t[:, a:b, U // 2 :])
            else:
                nc.sync.dma_start(out[:, a:b, :], acc_t[:, a:b, :])

    nc.compile()
    return nc


def _get_nc():
    if "nc" not in _NC_CACHE:
        _NC_CACHE["nc"] = _build_nc()
    return _NC_CACHE["nc"]


def kernel(x, g, alpha, beta, _trace=False, _trace_kwargs=None):
    global last_results
    import ml_dtypes

    bf16 = ml_dtypes.bfloat16
    x = np.asarray(x, dtype=np.float32)
    g = np.ascontiguousarray(np.asarray(g, dtype=np.float32))
    alpha = np.asarray(alpha, dtype=np.float32)
    beta = np.ascontiguousarray(np.asarray(beta, dtype=np.float32))

    # alpha[e,u,d] -> [E, P, K, U]: element (e, p, k, u) = alpha[e, u, k*P + p]
    alphaP = np.ascontiguousarray(
        alpha.transpose(0, 2, 1).reshape(E, K_TILES, P, U).transpose(0, 2, 1, 3)
    ).astype(bf16)
    xT = x.T.astype(bf16)  # [D, B]
    alpha0 = alphaP[0]  # [P, K, U]

    in_maps = []
    for c in range(N_CORES):
        sl = slice(c * BC, (c + 1) * BC)
        # xa0: [K, P, BC + U] = x^T k-chunk | alpha0 k-chunk
        xa0 = np.empty((K_TILES, P, BC + U), dtype=bf16)
        for k in range(K_TILES):
            xa0[k, :, :BC] = xT[k * P : (k + 1) * P, sl]
            xa0[k, :, BC:] = alpha0[:, k, :]
        gc = g[sl].reshape(M_TILES, P, E).transpose(1, 0, 2)
        bias = (g[sl] @ beta).astype(bf16)
        bc = bias.reshape(M_TILES, P, U).transpose(1, 0, 2)
        in_maps.append(
            {
                "xa0": np.ascontiguousarray(xa0),
                "gP": np.ascontiguousarray(gc),  # [P, M, E] f32
                "biasP": np.ascontiguousarray(bc),  # [P, M, U] bf16
                "alphaP": alphaP,  # [E, P, K, U] bf16 (replicated)
            }
        )

    nc = _get_nc()
    res = run_bass_kernel_spmd(
        nc,
        in_maps,
        list(range(N_CORES)),
        trace=_trace,
        **(_trace_kwargs or {}),
    )
    last_results = res
    return np.concatenate(
        [
            np.asarray(r["out"])
            .astype(np.float32)
            .transpose(1, 0, 2)
            .reshape(BC, U)
            for r in res.results
        ],
        axis=0,
    )
